# revision 18
# baseline (speedup 1.0000x reference)
"""Bahdanau attention TRN2 kernel: 8-core data-parallel over batch.

Reference computation (B=64, S=2048, H=U=1024, fp32):
    q_proj = query @ W1 + b1                     # [B,U]
    v_proj = values @ W2 + b2                    # [B,S,U]
    score  = tanh(q_proj[:,None,:] + v_proj) @ V + bv   # [B,S,1]
    attn   = softmax(score, axis=1)              # [B,S,1]
    ctx    = sum(attn * values, axis=1)          # [B,H]
    returns (ctx, attn)

Per-core strategy (8 examples each, software-pipelined):
  - host pre-transposes values -> valT [B, H, S] (bf16) so the device streams
    it straight into the TensorEngine as the moving operand; contraction over
    H needs H on partitions for both operands.
  - v_projT tile [u_tile=128, s] = sum_k W2[k,u].T @ valT[k,s], bf16 matmuls
    (1 cyc/row, FWL weight loads), fp32 psum.
  - ScalarE evacuates psum with fused tanh + per-partition bias
    (q_projT + b1 + b2), bf16 out.
  - score = V.T @ tanh on PE -> psum row; ScalarE exp with accumulated row
    sum (no max subtraction needed: scores are O(1); softmax shift-invariance
    also lets us drop bv entirely).
  - attn = exp * (1/sum); context on DVE: ctx[h] = sum_s valT[h,s]*attn[s]
    with attn broadcast across partitions by gpsimd partition_broadcast.
  - example b's softmax/context tail is emitted inside example b+1's body so
    the TensorEngine never stalls on the DVE-side chain.
"""

import sys
import types
import functools
import contextlib

if "/opt/trn_rl_repo" not in sys.path:
    sys.path.insert(0, "/opt/trn_rl_repo")

import numpy as np
import ml_dtypes

import concourse.bass as bass
import concourse.tile as tile
from concourse import mybir, bacc
from concourse.bass_utils import run_bass_kernel_spmd
from concourse.masks import make_identity

N_CORES = 8
B, S, H, U = 64, 2048, 1024, 1024
BPC = B // N_CORES          # examples per core
P = 128                     # partitions
HC = H // P                 # h-chunks
UC = U // P                 # u-chunks
ST = 512                    # s-tile width (one fp32 psum bank)
NS = S // ST
F32 = mybir.dt.float32
BF16 = mybir.dt.bfloat16
AF = mybir.ActivationFunctionType
ALU = mybir.AluOpType
BF = ml_dtypes.bfloat16


def _install_ntff_shim():
    """antenv.axon_hooks is absent in this image; recreate it so
    run_bass_kernel_spmd(trace=True) can capture NTFF profiles."""
    try:
        from antenv import axon_hooks  # noqa: F401
        return
    except ImportError:
        pass
    import antenv
    mod = types.ModuleType("antenv.axon_hooks")
    mod._hook = None
    def set_axon_ntff_profile_hook(h):
        mod._hook = h
    def get_axon_ntff_profile_hook():
        return mod._hook
    mod.set_axon_ntff_profile_hook = set_axon_ntff_profile_hook
    mod.get_axon_ntff_profile_hook = get_axon_ntff_profile_hook
    sys.modules["antenv.axon_hooks"] = mod
    antenv.axon_hooks = mod
    try:
        from trn_agent_boot.trn_boot import _ntff_profile_via_ctypes
        set_axon_ntff_profile_hook(
            _ntff_profile_via_ctypes("/opt/axon/libaxon_pjrt.so")
        )
    except Exception:
        pass


@functools.lru_cache(maxsize=2)
def _build_nc():
    nc = bacc.Bacc("TRN2", target_bir_lowering=False, debug=False)

    qT = nc.declare_dram_parameter("qT", [H, BPC], BF16, isOutput=False)
    valT = nc.declare_dram_parameter("valT", [BPC, H, S], BF16, isOutput=False)
    w1 = nc.declare_dram_parameter("w1", [H, U], BF16, isOutput=False)
    w2 = nc.declare_dram_parameter("w2", [H, U], BF16, isOutput=False)
    b12t = nc.declare_dram_parameter("b12t", [P, UC], F32, isOutput=False)
    vt = nc.declare_dram_parameter("vt", [P, UC], BF16, isOutput=False)
    ctx_out = nc.declare_dram_parameter("ctx", [BPC, H], F32, isOutput=True)
    attn_out = nc.declare_dram_parameter("attn", [BPC, S], F32, isOutput=True)

    with tile.TileContext(nc) as tc, contextlib.ExitStack() as ex:
        consts = ex.enter_context(tc.tile_pool(name="consts", bufs=1))
        w2p = ex.enter_context(tc.tile_pool(name="w2p", bufs=1))
        valp = ex.enter_context(tc.tile_pool(name="valp", bufs=2 * HC))
        tanhp = ex.enter_context(tc.tile_pool(name="tanhp", bufs=8))
        junkp = ex.enter_context(tc.tile_pool(name="junkp", bufs=1))
        scoresp = ex.enter_context(tc.tile_pool(name="scoresp", bufs=2))
        bcp = ex.enter_context(tc.tile_pool(name="bcp", bufs=4))
        smallp = ex.enter_context(tc.tile_pool(name="smallp", bufs=2))
        psum_mm = ex.enter_context(tc.tile_pool(name="psum_mm", bufs=4, space="PSUM"))
        psum_sc = ex.enter_context(tc.tile_pool(name="psum_sc", bufs=3, space="PSUM"))

        identb = consts.tile([P, P], BF16)
        make_identity(nc, identb)
        b12sb = consts.tile([P, UC], F32)
        nc.sync.dma_start(out=b12sb, in_=b12t.ap())
        vtsb = consts.tile([P, UC], BF16)
        nc.sync.dma_start(out=vtsb, in_=vt.ap())
        qsb = consts.tile([P, HC, BPC], BF16)
        nc.sync.dma_start(out=qsb, in_=qT.ap().rearrange("(hc p) b -> p hc b", p=P))

        # q_projT[u, b] (+ b1 + b2), laid out [P, UC, BPC]; fp32, startup-only.
        qproj = consts.tile([P, UC, BPC], F32)
        with tc.tile_pool(name="w1p", bufs=1) as w1p:
            w1sb = w1p.tile([P, HC, U], BF16)
            nc.sync.dma_start(
                out=w1sb, in_=w1.ap().rearrange("(hc p) u -> p hc u", p=P)
            )
            for m in range(UC):
                pq = psum_mm.tile([P, BPC], F32, tag="mm")
                for k in range(HC):
                    nc.tensor.matmul(
                        pq,
                        lhsT=w1sb[:, k, m * P : (m + 1) * P],
                        rhs=qsb[:, k, :],
                        start=(k == 0),
                        stop=(k == HC - 1),
                    )
                nc.vector.tensor_scalar_add(qproj[:, m, :], pq, b12sb[:, m : m + 1])

        w2sb = w2p.tile([P, HC, U], BF16)
        nc.sync.dma_start(out=w2sb, in_=w2.ap().rearrange("(hc p) u -> p hc u", p=P))

        def emit_loads(b):
            vts = []
            for k in range(HC):
                t = valp.tile([P, S], BF16, tag="valT")
                nc.sync.dma_start(out=t, in_=valT.ap()[b, k * P : (k + 1) * P, :])
                vts.append(t)
            return vts

        def emit_exp(ps, si, sc, lsum):
            nc.scalar.activation(
                out=sc[:, si * ST : (si + 1) * ST], in_=ps, func=AF.Exp,
                accum_out=lsum[:, si : si + 1],
            )

        def emit_compute_si(b, si, vts, sc, lsum):
            s0 = si * ST
            tts = []
            ps = psum_sc.tile([1, ST], F32, tag="sc")

            def score_mm(m):
                nc.tensor.matmul(
                    ps, lhsT=vtsb[:, m : m + 1], rhs=tts[m],
                    start=(m == 0), stop=(m == UC - 1),
                )

            for m in range(UC):
                pv = psum_mm.tile([P, ST], F32, tag="mm")
                for k in range(HC):
                    nc.tensor.matmul(
                        pv,
                        lhsT=w2sb[:, k, m * P : (m + 1) * P],
                        rhs=vts[k][:, s0 : s0 + ST],
                        start=(k == 0),
                        stop=(k == HC - 1),
                    )
                th = tanhp.tile([P, ST], BF16, tag="tanh")
                nc.scalar.activation(
                    out=th, in_=pv, func=AF.Tanh,
                    bias=qproj[:, m, b : b + 1], scale=1.0,
                )
                tts.append(th)
                # interleave score matmuls two m-tiles behind so their tanh
                # inputs are already evacuated — PE never waits on ScalarE
                if m >= 2:
                    score_mm(m - 2)
            score_mm(UC - 2)
            score_mm(UC - 1)
            return ps

        def emit_tail_a(b, vts, sc, lsum, last=False):
            """softmax + attn out + bcast + ctx accumulate (no PE)."""
            ltot = smallp.tile([1, 1], F32, tag="ltot")
            nc.vector.reduce_sum(out=ltot, in_=lsum, axis=mybir.AxisListType.X)
            rinv = smallp.tile([1, 1], F32, tag="rinv")
            nc.vector.reciprocal(rinv, ltot)
            nc.vector.tensor_scalar_mul(sc, sc, rinv)
            nc.gpsimd.dma_start(out=attn_out.ap()[b : b + 1, :], in_=sc)
            bcs = []
            for q in range(NS):
                bc = bcp.tile([P, ST], BF16, tag="bc")
                nc.gpsimd.partition_broadcast(bc, sc[:, q * ST : (q + 1) * ST])
                bcs.append(bc)
            cacc = smallp.tile([P, HC], F32, tag="cacc")
            junk = junkp.tile([P, S], BF16, tag="junk")
            junk2 = junkp.tile([P, S], BF16, tag="junk2")
            junk3 = junkp.tile([P, S], BF16, tag="junk3")
            for k in range(HC):
                jk = junk if k % 2 == 0 else junk2
                for q in range(NS):
                    eng = nc.gpsimd if (last and (k * NS + q) % 8 >= 5) else nc.vector
                    eng.tensor_mul(
                        jk[:, q * ST : (q + 1) * ST],
                        vts[k][:, q * ST : (q + 1) * ST],
                        bcs[q],
                    )
                if last and k % 2 == 0:
                    # final tail overlaps nothing: split reduces onto ScalarE
                    nc.scalar.activation(
                        out=junk3, in_=jk, func=AF.Identity,
                        accum_out=cacc[:, k : k + 1],
                    )
                else:
                    nc.vector.reduce_sum(
                        out=cacc[:, k : k + 1], in_=jk, axis=mybir.AxisListType.X
                    )
            return cacc

        def emit_tail_b(b, cacc):
            """ctx transpose + writeback (one tiny bf16 PE op, placed late)."""
            caccb = smallp.tile([P, HC], BF16, tag="caccb")
            nc.vector.tensor_copy(caccb, cacc)
            pt = psum_sc.tile([HC, P], BF16, tag="sc")
            nc.tensor.transpose(pt, caccb, identb)
            ctxT = smallp.tile([HC, P], BF16, tag="ctxT")
            nc.vector.tensor_copy(ctxT, pt)
            nc.gpsimd.dma_start(
                out=ctx_out.ap()[b : b + 1, :].rearrange(
                    "one (hc p) -> hc (one p)", p=P
                ),
                in_=ctxT,
            )

        # software pipeline: example b's tail is emitted inside example b+1's
        # body so PE never waits on the DVE-side softmax/context chain.
        prev = None
        prev_cacc = None
        pend_exp = None      # (ps, si, sc, lsum) — exp deferred one s-tile
        for b in range(BPC):
            vts = emit_loads(b)
            sc = scoresp.tile([1, S], BF16, tag="scores")
            lsum = smallp.tile([1, NS], F32, tag="lsum")
            for si in range(NS):
                ps = emit_compute_si(b, si, vts, sc, lsum)
                if pend_exp is not None:
                    emit_exp(*pend_exp)
                pend_exp = (ps, si, sc, lsum)
                if si == 0 and prev is not None:
                    prev_cacc = emit_tail_a(*prev)
            if prev is not None:
                emit_tail_b(prev[0], prev_cacc)
            prev = (b, vts, sc, lsum)
        emit_exp(*pend_exp)
        pend_exp = None
        prev_cacc = emit_tail_a(*prev, last=True)
        emit_tail_b(prev[0], prev_cacc)

    nc.compile()
    return nc


def _make_in_maps(query, values, W1, b1, W2, b2, V):
    # bf16 data path for the big operands; fp32 for the bias/query path.
    valT = np.ascontiguousarray(
        values.transpose(0, 2, 1).astype(BF)
    )  # [B, H, S] bf16
    b12 = (b1 + b2).astype(np.float32)
    b12t = np.ascontiguousarray(b12.reshape(UC, P).T)
    vt = np.ascontiguousarray(V.reshape(UC, P).T).astype(BF)
    w1 = np.ascontiguousarray(W1.astype(BF))
    w2 = np.ascontiguousarray(W2.astype(BF))
    in_maps = []
    for c in range(N_CORES):
        sl = slice(c * BPC, (c + 1) * BPC)
        in_maps.append(
            {
                "qT": np.ascontiguousarray(query[sl].T).astype(BF),
                "valT": np.ascontiguousarray(valT[sl]),
                "w1": w1,
                "w2": w2,
                "b12t": b12t,
                "vt": vt,
            }
        )
    return in_maps


def _run(in_maps, trace=False, tmpdir=None):
    if trace:
        _install_ntff_shim()
    nc = _build_nc()
    res = run_bass_kernel_spmd(
        nc, in_maps, core_ids=list(range(N_CORES)), trace=trace, tmpdir=tmpdir
    )
    ctx = np.concatenate([res.results[c]["ctx"] for c in range(N_CORES)], axis=0)
    attn = np.concatenate([res.results[c]["attn"] for c in range(N_CORES)], axis=0)
    return (ctx, attn.reshape(B, S, 1)), res


def kernel(query, values, W1, b1, W2, b2, V, bv):
    # bv shifts every score of an example equally; softmax is shift-invariant,
    # so it affects neither output and is dropped.
    in_maps = _make_in_maps(
        np.asarray(query), np.asarray(values), np.asarray(W1), np.asarray(b1),
        np.asarray(W2), np.asarray(b2), np.asarray(V)
    )
    outs, _ = _run(in_maps, trace=False)
    return outs


# revision 19
# speedup vs baseline: 1.0696x; 1.0696x over previous
"""Bahdanau attention TRN2 kernel: 8-core data-parallel over batch.

Reference computation (B=64, S=2048, H=U=1024, fp32):
    q_proj = query @ W1 + b1                     # [B,U]
    v_proj = values @ W2 + b2                    # [B,S,U]
    score  = tanh(q_proj[:,None,:] + v_proj) @ V + bv   # [B,S,1]
    attn   = softmax(score, axis=1)              # [B,S,1]
    ctx    = sum(attn * values, axis=1)          # [B,H]
    returns (ctx, attn)

Per-core strategy (8 examples each, software-pipelined):
  - host pre-transposes values -> valT [B, H, S] (bf16) so the device streams
    it straight into the TensorEngine as the moving operand; contraction over
    H needs H on partitions for both operands.
  - v_projT tile [u_tile=128, s] = sum_k W2[k,u].T @ valT[k,s], bf16 matmuls
    (1 cyc/row, FWL weight loads), fp32 psum.
  - ScalarE evacuates psum with fused tanh + per-partition bias
    (q_projT + b1 + b2), bf16 out.
  - score = V.T @ tanh on PE -> psum row; ScalarE exp with accumulated row
    sum (no max subtraction needed: scores are O(1); softmax shift-invariance
    also lets us drop bv entirely).
  - attn = exp * (1/sum); context on DVE: ctx[h] = sum_s valT[h,s]*attn[s]
    with attn broadcast across partitions by gpsimd partition_broadcast.
  - example b's softmax/context tail is emitted inside example b+1's body so
    the TensorEngine never stalls on the DVE-side chain.
"""

import sys
import types
import functools
import contextlib

if "/opt/trn_rl_repo" not in sys.path:
    sys.path.insert(0, "/opt/trn_rl_repo")

import numpy as np
import ml_dtypes

import concourse.bass as bass
import concourse.tile as tile
from concourse import mybir, bacc
from concourse.bass_utils import run_bass_kernel_spmd
from concourse.masks import make_identity

N_CORES = 8
B, S, H, U = 64, 2048, 1024, 1024
BPC = B // N_CORES          # examples per core
P = 128                     # partitions
HC = H // P                 # h-chunks
UC = U // P                 # u-chunks
ST = 512                    # s-tile width (one fp32 psum bank)
NS = S // ST
F32 = mybir.dt.float32
BF16 = mybir.dt.bfloat16
AF = mybir.ActivationFunctionType
ALU = mybir.AluOpType
BF = ml_dtypes.bfloat16


def _install_ntff_shim():
    """antenv.axon_hooks is absent in this image; recreate it so
    run_bass_kernel_spmd(trace=True) can capture NTFF profiles."""
    try:
        from antenv import axon_hooks  # noqa: F401
        return
    except ImportError:
        pass
    import antenv
    mod = types.ModuleType("antenv.axon_hooks")
    mod._hook = None
    def set_axon_ntff_profile_hook(h):
        mod._hook = h
    def get_axon_ntff_profile_hook():
        return mod._hook
    mod.set_axon_ntff_profile_hook = set_axon_ntff_profile_hook
    mod.get_axon_ntff_profile_hook = get_axon_ntff_profile_hook
    sys.modules["antenv.axon_hooks"] = mod
    antenv.axon_hooks = mod
    try:
        from trn_agent_boot.trn_boot import _ntff_profile_via_ctypes
        set_axon_ntff_profile_hook(
            _ntff_profile_via_ctypes("/opt/axon/libaxon_pjrt.so")
        )
    except Exception:
        pass


@functools.lru_cache(maxsize=2)
def _build_nc():
    nc = bacc.Bacc("TRN2", target_bir_lowering=False, debug=False)

    qT = nc.declare_dram_parameter("qT", [H, BPC], BF16, isOutput=False)
    valT = nc.declare_dram_parameter("valT", [BPC, H, S], BF16, isOutput=False)
    w1 = nc.declare_dram_parameter("w1", [H, U], BF16, isOutput=False)
    w2 = nc.declare_dram_parameter("w2", [H, U], BF16, isOutput=False)
    b12t = nc.declare_dram_parameter("b12t", [P, UC], F32, isOutput=False)
    vt = nc.declare_dram_parameter("vt", [P, UC], BF16, isOutput=False)
    ctx_out = nc.declare_dram_parameter("ctx", [BPC, H], F32, isOutput=True)
    attn_out = nc.declare_dram_parameter("attn", [BPC, S], F32, isOutput=True)

    with tile.TileContext(nc) as tc, contextlib.ExitStack() as ex:
        consts = ex.enter_context(tc.tile_pool(name="consts", bufs=1))
        w2p = ex.enter_context(tc.tile_pool(name="w2p", bufs=1))
        valp = ex.enter_context(tc.tile_pool(name="valp", bufs=2 * HC))
        tanhp = ex.enter_context(tc.tile_pool(name="tanhp", bufs=8))
        junkp = ex.enter_context(tc.tile_pool(name="junkp", bufs=1))
        scoresp = ex.enter_context(tc.tile_pool(name="scoresp", bufs=2))
        bcp = ex.enter_context(tc.tile_pool(name="bcp", bufs=4))
        smallp = ex.enter_context(tc.tile_pool(name="smallp", bufs=2))
        psum_mm = ex.enter_context(tc.tile_pool(name="psum_mm", bufs=4, space="PSUM"))
        psum_sc = ex.enter_context(tc.tile_pool(name="psum_sc", bufs=3, space="PSUM"))

        identb = consts.tile([P, P], BF16)
        make_identity(nc, identb)
        b12sb = consts.tile([P, UC], F32)
        nc.sync.dma_start(out=b12sb, in_=b12t.ap())
        vtsb = consts.tile([P, UC], BF16)
        nc.sync.dma_start(out=vtsb, in_=vt.ap())
        qsb = consts.tile([P, HC, BPC], BF16)
        nc.sync.dma_start(out=qsb, in_=qT.ap().rearrange("(hc p) b -> p hc b", p=P))

        # q_projT[u, b] (+ b1 + b2), laid out [P, UC, BPC]; fp32, startup-only.
        qproj = consts.tile([P, UC, BPC], F32)
        with tc.tile_pool(name="w1p", bufs=1) as w1p:
            w1sb = w1p.tile([P, HC, U], BF16)
            nc.sync.dma_start(
                out=w1sb, in_=w1.ap().rearrange("(hc p) u -> p hc u", p=P)
            )
            for m in range(UC):
                pq = psum_mm.tile([P, BPC], F32, tag="mm")
                for k in range(HC):
                    nc.tensor.matmul(
                        pq,
                        lhsT=w1sb[:, k, m * P : (m + 1) * P],
                        rhs=qsb[:, k, :],
                        start=(k == 0),
                        stop=(k == HC - 1),
                    )
                nc.vector.tensor_scalar_add(qproj[:, m, :], pq, b12sb[:, m : m + 1])

        w2sb = w2p.tile([P, HC, U], BF16)
        nc.sync.dma_start(out=w2sb, in_=w2.ap().rearrange("(hc p) u -> p hc u", p=P))

        def emit_loads(b):
            vts = []
            for k in range(HC):
                t = valp.tile([P, S], BF16, tag="valT")
                nc.sync.dma_start(out=t, in_=valT.ap()[b, k * P : (k + 1) * P, :])
                vts.append(t)
            return vts

        def emit_exp(ps, si, sc, lsum):
            nc.scalar.activation(
                out=sc[:, si * ST : (si + 1) * ST], in_=ps, func=AF.Exp,
                accum_out=lsum[:, si : si + 1],
            )

        def emit_compute_si(b, si, vts, sc, lsum):
            s0 = si * ST
            tts = []
            for m in range(UC):
                pv = psum_mm.tile([P, ST], F32, tag="mm")
                for k in range(HC):
                    nc.tensor.matmul(
                        pv,
                        lhsT=w2sb[:, k, m * P : (m + 1) * P],
                        rhs=vts[k][:, s0 : s0 + ST],
                        start=(k == 0),
                        stop=(k == HC - 1),
                    )
                th = tanhp.tile([P, ST], BF16, tag="tanh")
                nc.scalar.activation(
                    out=th, in_=pv, func=AF.Tanh,
                    bias=qproj[:, m, b : b + 1], scale=1.0,
                )
                tts.append(th)
            ps = psum_sc.tile([1, ST], F32, tag="sc")
            for m in range(UC):
                nc.tensor.matmul(
                    ps, lhsT=vtsb[:, m : m + 1], rhs=tts[m],
                    start=(m == 0), stop=(m == UC - 1),
                )
            return ps

        def emit_tail_a(b, vts, sc, lsum, last=False):
            """softmax + attn out + bcast + ctx accumulate (no PE)."""
            ltot = smallp.tile([1, 1], F32, tag="ltot")
            nc.vector.reduce_sum(out=ltot, in_=lsum, axis=mybir.AxisListType.X)
            rinv = smallp.tile([1, 1], F32, tag="rinv")
            nc.vector.reciprocal(rinv, ltot)
            nc.vector.tensor_scalar_mul(sc, sc, rinv)
            nc.gpsimd.dma_start(out=attn_out.ap()[b : b + 1, :], in_=sc)
            bcs = []
            for q in range(NS):
                bc = bcp.tile([P, ST], BF16, tag="bc")
                nc.gpsimd.partition_broadcast(bc, sc[:, q * ST : (q + 1) * ST])
                bcs.append(bc)
            cacc = smallp.tile([P, HC], F32, tag="cacc")
            junk = junkp.tile([P, S], BF16, tag="junk")
            junk2 = junkp.tile([P, S], BF16, tag="junk2")
            junk3 = junkp.tile([P, S], BF16, tag="junk3")
            for k in range(HC):
                jk = junk if k % 2 == 0 else junk2
                for q in range(NS):
                    eng = nc.gpsimd if (last and (k * NS + q) % 8 >= 5) else nc.vector
                    eng.tensor_mul(
                        jk[:, q * ST : (q + 1) * ST],
                        vts[k][:, q * ST : (q + 1) * ST],
                        bcs[q],
                    )
                if last and k % 2 == 0:
                    # final tail overlaps nothing: split reduces onto ScalarE
                    nc.scalar.activation(
                        out=junk3, in_=jk, func=AF.Identity,
                        accum_out=cacc[:, k : k + 1],
                    )
                else:
                    nc.vector.reduce_sum(
                        out=cacc[:, k : k + 1], in_=jk, axis=mybir.AxisListType.X
                    )
            return cacc

        def emit_tail_b(b, cacc):
            """ctx transpose + writeback (one tiny bf16 PE op, placed late)."""
            caccb = smallp.tile([P, HC], BF16, tag="caccb")
            nc.vector.tensor_copy(caccb, cacc)
            pt = psum_sc.tile([HC, P], BF16, tag="sc")
            nc.tensor.transpose(pt, caccb, identb)
            ctxT = smallp.tile([HC, P], BF16, tag="ctxT")
            nc.vector.tensor_copy(ctxT, pt)
            nc.gpsimd.dma_start(
                out=ctx_out.ap()[b : b + 1, :].rearrange(
                    "one (hc p) -> hc (one p)", p=P
                ),
                in_=ctxT,
            )

        # software pipeline: example b's tail is emitted inside example b+1's
        # body so PE never waits on the DVE-side softmax/context chain.
        prev = None
        prev_cacc = None
        pend_exp = None      # (ps, si, sc, lsum) — exp deferred one s-tile
        for b in range(BPC):
            vts = emit_loads(b)
            sc = scoresp.tile([1, S], BF16, tag="scores")
            lsum = smallp.tile([1, NS], F32, tag="lsum")
            for si in range(NS):
                ps = emit_compute_si(b, si, vts, sc, lsum)
                if pend_exp is not None:
                    emit_exp(*pend_exp)
                pend_exp = (ps, si, sc, lsum)
                if si == 0 and prev is not None:
                    prev_cacc = emit_tail_a(*prev)
            if prev is not None:
                emit_tail_b(prev[0], prev_cacc)
            prev = (b, vts, sc, lsum)
        emit_exp(*pend_exp)
        pend_exp = None
        prev_cacc = emit_tail_a(*prev, last=True)
        emit_tail_b(prev[0], prev_cacc)

    nc.compile()
    return nc


def _make_in_maps(query, values, W1, b1, W2, b2, V):
    # bf16 data path for the big operands; fp32 for the bias/query path.
    valT = np.ascontiguousarray(
        values.transpose(0, 2, 1).astype(BF)
    )  # [B, H, S] bf16
    b12 = (b1 + b2).astype(np.float32)
    b12t = np.ascontiguousarray(b12.reshape(UC, P).T)
    vt = np.ascontiguousarray(V.reshape(UC, P).T).astype(BF)
    w1 = np.ascontiguousarray(W1.astype(BF))
    w2 = np.ascontiguousarray(W2.astype(BF))
    in_maps = []
    for c in range(N_CORES):
        sl = slice(c * BPC, (c + 1) * BPC)
        in_maps.append(
            {
                "qT": np.ascontiguousarray(query[sl].T).astype(BF),
                "valT": np.ascontiguousarray(valT[sl]),
                "w1": w1,
                "w2": w2,
                "b12t": b12t,
                "vt": vt,
            }
        )
    return in_maps


def _run(in_maps, trace=False, tmpdir=None):
    if trace:
        _install_ntff_shim()
    nc = _build_nc()
    res = run_bass_kernel_spmd(
        nc, in_maps, core_ids=list(range(N_CORES)), trace=trace, tmpdir=tmpdir
    )
    ctx = np.concatenate([res.results[c]["ctx"] for c in range(N_CORES)], axis=0)
    attn = np.concatenate([res.results[c]["attn"] for c in range(N_CORES)], axis=0)
    return (ctx, attn.reshape(B, S, 1)), res


def kernel(query, values, W1, b1, W2, b2, V, bv):
    # bv shifts every score of an example equally; softmax is shift-invariant,
    # so it affects neither output and is dropped.
    in_maps = _make_in_maps(
        np.asarray(query), np.asarray(values), np.asarray(W1), np.asarray(b1),
        np.asarray(W2), np.asarray(b2), np.asarray(V)
    )
    outs, _ = _run(in_maps, trace=False)
    return outs


# revision 20
# speedup vs baseline: 1.0742x; 1.0043x over previous
"""Bahdanau attention TRN2 kernel: 8-core data-parallel over batch.

Reference computation (B=64, S=2048, H=U=1024, fp32):
    q_proj = query @ W1 + b1                     # [B,U]
    v_proj = values @ W2 + b2                    # [B,S,U]
    score  = tanh(q_proj[:,None,:] + v_proj) @ V + bv   # [B,S,1]
    attn   = softmax(score, axis=1)              # [B,S,1]
    ctx    = sum(attn * values, axis=1)          # [B,H]
    returns (ctx, attn)

Per-core strategy (8 examples each, software-pipelined):
  - host pre-transposes values -> valT [B, H, S] (bf16) so the device streams
    it straight into the TensorEngine as the moving operand; contraction over
    H needs H on partitions for both operands.
  - v_projT tile [u_tile=128, s] = sum_k W2[k,u].T @ valT[k,s], bf16 matmuls
    (1 cyc/row, FWL weight loads), fp32 psum.
  - ScalarE evacuates psum with fused tanh + per-partition bias
    (q_projT + b1 + b2), bf16 out.
  - score = V.T @ tanh on PE -> psum row; ScalarE exp with accumulated row
    sum (no max subtraction needed: scores are O(1); softmax shift-invariance
    also lets us drop bv entirely).
  - attn = exp * (1/sum); context on DVE: ctx[h] = sum_s valT[h,s]*attn[s]
    with attn broadcast across partitions by gpsimd partition_broadcast.
  - example b's softmax/context tail is emitted inside example b+1's body so
    the TensorEngine never stalls on the DVE-side chain.
"""

import sys
import types
import functools
import contextlib

if "/opt/trn_rl_repo" not in sys.path:
    sys.path.insert(0, "/opt/trn_rl_repo")

import numpy as np
import ml_dtypes

import concourse.bass as bass
import concourse.tile as tile
from concourse import mybir, bacc
from concourse.bass_utils import run_bass_kernel_spmd
from concourse.masks import make_identity

N_CORES = 8
B, S, H, U = 64, 2048, 1024, 1024
BPC = B // N_CORES          # examples per core
P = 128                     # partitions
HC = H // P                 # h-chunks
UC = U // P                 # u-chunks
ST = 512                    # s-tile width (one fp32 psum bank)
NS = S // ST
F32 = mybir.dt.float32
BF16 = mybir.dt.bfloat16
AF = mybir.ActivationFunctionType
ALU = mybir.AluOpType
BF = ml_dtypes.bfloat16


def _install_ntff_shim():
    """antenv.axon_hooks is absent in this image; recreate it so
    run_bass_kernel_spmd(trace=True) can capture NTFF profiles."""
    try:
        from antenv import axon_hooks  # noqa: F401
        return
    except ImportError:
        pass
    import antenv
    mod = types.ModuleType("antenv.axon_hooks")
    mod._hook = None
    def set_axon_ntff_profile_hook(h):
        mod._hook = h
    def get_axon_ntff_profile_hook():
        return mod._hook
    mod.set_axon_ntff_profile_hook = set_axon_ntff_profile_hook
    mod.get_axon_ntff_profile_hook = get_axon_ntff_profile_hook
    sys.modules["antenv.axon_hooks"] = mod
    antenv.axon_hooks = mod
    try:
        from trn_agent_boot.trn_boot import _ntff_profile_via_ctypes
        set_axon_ntff_profile_hook(
            _ntff_profile_via_ctypes("/opt/axon/libaxon_pjrt.so")
        )
    except Exception:
        pass


@functools.lru_cache(maxsize=2)
def _build_nc():
    nc = bacc.Bacc("TRN2", target_bir_lowering=False, debug=False)

    qT = nc.declare_dram_parameter("qT", [H, BPC], BF16, isOutput=False)
    valT = nc.declare_dram_parameter("valT", [BPC, H, S], BF16, isOutput=False)
    w1 = nc.declare_dram_parameter("w1", [H, U], BF16, isOutput=False)
    w2 = nc.declare_dram_parameter("w2", [H, U], BF16, isOutput=False)
    b12t = nc.declare_dram_parameter("b12t", [P, UC], F32, isOutput=False)
    vt = nc.declare_dram_parameter("vt", [P, UC, P], BF16, isOutput=False)
    ctx_out = nc.declare_dram_parameter("ctx", [BPC, H], F32, isOutput=True)
    attn_out = nc.declare_dram_parameter("attn", [BPC, S], F32, isOutput=True)

    with tile.TileContext(nc) as tc, contextlib.ExitStack() as ex:
        consts = ex.enter_context(tc.tile_pool(name="consts", bufs=1))
        w2p = ex.enter_context(tc.tile_pool(name="w2p", bufs=1))
        valp = ex.enter_context(tc.tile_pool(name="valp", bufs=2 * HC))
        tanhp = ex.enter_context(tc.tile_pool(name="tanhp", bufs=8))
        junkp = ex.enter_context(tc.tile_pool(name="junkp", bufs=1))
        scoresp = ex.enter_context(tc.tile_pool(name="scoresp", bufs=2))
        bcp = ex.enter_context(tc.tile_pool(name="bcp", bufs=4))
        smallp = ex.enter_context(tc.tile_pool(name="smallp", bufs=2))
        psum_mm = ex.enter_context(tc.tile_pool(name="psum_mm", bufs=5, space="PSUM"))
        psum_sc = ex.enter_context(tc.tile_pool(name="psum_sc", bufs=3, space="PSUM"))

        identb = consts.tile([P, P], BF16)
        make_identity(nc, identb)
        b12sb = consts.tile([P, UC], F32)
        nc.sync.dma_start(out=b12sb, in_=b12t.ap())
        vtsb = consts.tile([P, UC, P], BF16)
        nc.sync.dma_start(out=vtsb, in_=vt.ap())
        qsb = consts.tile([P, HC, BPC], BF16)
        nc.sync.dma_start(out=qsb, in_=qT.ap().rearrange("(hc p) b -> p hc b", p=P))

        # q_projT[u, b] (+ b1 + b2), laid out [P, UC, BPC]; fp32, startup-only.
        qproj = consts.tile([P, UC, BPC], F32)
        with tc.tile_pool(name="w1p", bufs=1) as w1p:
            w1sb = w1p.tile([P, HC, U], BF16)
            nc.sync.dma_start(
                out=w1sb, in_=w1.ap().rearrange("(hc p) u -> p hc u", p=P)
            )
            for m in range(UC):
                pq = psum_mm.tile([P, BPC], F32, tag="mm")
                for k in range(HC):
                    nc.tensor.matmul(
                        pq,
                        lhsT=w1sb[:, k, m * P : (m + 1) * P],
                        rhs=qsb[:, k, :],
                        start=(k == 0),
                        stop=(k == HC - 1),
                    )
                nc.vector.tensor_scalar_add(qproj[:, m, :], pq, b12sb[:, m : m + 1])

        w2sb = w2p.tile([P, HC, U], BF16)
        nc.sync.dma_start(out=w2sb, in_=w2.ap().rearrange("(hc p) u -> p hc u", p=P))

        def emit_loads(b):
            vts = []
            for k in range(HC):
                t = valp.tile([P, S], BF16, tag="valT")
                nc.sync.dma_start(out=t, in_=valT.ap()[b, k * P : (k + 1) * P, :])
                vts.append(t)
            return vts

        def emit_exp(ps, si, sc, lsum):
            nc.scalar.activation(
                out=sc[:, si * ST : (si + 1) * ST], in_=ps[0:1, :], func=AF.Exp,
                accum_out=lsum[:, si : si + 1],
            )

        def emit_compute_si(b, si, vts, sc, lsum):
            s0 = si * ST
            tts = []
            for m in range(UC):
                pv = psum_mm.tile([P, ST], F32, tag="mm")
                for k in range(HC):
                    nc.tensor.matmul(
                        pv,
                        lhsT=w2sb[:, k, m * P : (m + 1) * P],
                        rhs=vts[k][:, s0 : s0 + ST],
                        start=(k == 0),
                        stop=(k == HC - 1),
                    )
                th = tanhp.tile([P, ST], BF16, tag="tanh")
                nc.scalar.activation(
                    out=th, in_=pv, func=AF.Tanh,
                    bias=qproj[:, m, b : b + 1], scale=1.0,
                )
                tts.append(th)
            ps = psum_sc.tile([P, ST], F32, tag="sc")
            for m in range(UC):
                nc.tensor.matmul(
                    ps, lhsT=vtsb[:, m, :], rhs=tts[m],
                    start=(m == 0), stop=(m == UC - 1),
                )
            return ps

        def emit_tail_a(b, vts, sc, lsum, last=False):
            """softmax + attn out + bcast + ctx accumulate (no PE)."""
            ltot = smallp.tile([1, 1], F32, tag="ltot")
            nc.vector.reduce_sum(out=ltot, in_=lsum, axis=mybir.AxisListType.X)
            rinv = smallp.tile([1, 1], F32, tag="rinv")
            nc.vector.reciprocal(rinv, ltot)
            nc.vector.tensor_scalar_mul(sc, sc, rinv)
            nc.gpsimd.dma_start(out=attn_out.ap()[b : b + 1, :], in_=sc)
            bcs = []
            for q in range(NS):
                bc = bcp.tile([P, ST], BF16, tag="bc")
                nc.gpsimd.partition_broadcast(bc, sc[:, q * ST : (q + 1) * ST])
                bcs.append(bc)
            cacc = smallp.tile([P, HC], F32, tag="cacc")
            junk = junkp.tile([P, S], BF16, tag="junk")
            junk2 = junkp.tile([P, S], BF16, tag="junk2")
            junk3 = junkp.tile([P, S], BF16, tag="junk3")
            for k in range(HC):
                jk = junk if k % 2 == 0 else junk2
                for q in range(NS):
                    eng = nc.gpsimd if (last and (k * NS + q) % 8 >= 5) else nc.vector
                    eng.tensor_mul(
                        jk[:, q * ST : (q + 1) * ST],
                        vts[k][:, q * ST : (q + 1) * ST],
                        bcs[q],
                    )
                if last and k % 2 == 0:
                    # final tail overlaps nothing: split reduces onto ScalarE
                    nc.scalar.activation(
                        out=junk3, in_=jk, func=AF.Identity,
                        accum_out=cacc[:, k : k + 1],
                    )
                else:
                    nc.vector.reduce_sum(
                        out=cacc[:, k : k + 1], in_=jk, axis=mybir.AxisListType.X
                    )
            return cacc

        def emit_tail_b(b, cacc):
            """ctx transpose + writeback (one tiny bf16 PE op, placed late)."""
            caccb = smallp.tile([P, HC], BF16, tag="caccb")
            nc.vector.tensor_copy(caccb, cacc)
            pt = psum_sc.tile([HC, P], BF16, tag="sc")
            nc.tensor.transpose(pt, caccb, identb)
            ctxT = smallp.tile([HC, P], BF16, tag="ctxT")
            nc.vector.tensor_copy(ctxT, pt)
            nc.gpsimd.dma_start(
                out=ctx_out.ap()[b : b + 1, :].rearrange(
                    "one (hc p) -> hc (one p)", p=P
                ),
                in_=ctxT,
            )

        # software pipeline: example b's tail is emitted inside example b+1's
        # body so PE never waits on the DVE-side softmax/context chain.
        prev = None
        prev_cacc = None
        pend_exp = None      # (ps, si, sc, lsum) — exp deferred one s-tile
        for b in range(BPC):
            vts = emit_loads(b)
            sc = scoresp.tile([1, S], BF16, tag="scores")
            lsum = smallp.tile([1, NS], F32, tag="lsum")
            for si in range(NS):
                ps = emit_compute_si(b, si, vts, sc, lsum)
                if pend_exp is not None:
                    emit_exp(*pend_exp)
                pend_exp = (ps, si, sc, lsum)
                if si == 0 and prev is not None:
                    prev_cacc = emit_tail_a(*prev)
            if prev is not None:
                emit_tail_b(prev[0], prev_cacc)
            prev = (b, vts, sc, lsum)
        emit_exp(*pend_exp)
        pend_exp = None
        prev_cacc = emit_tail_a(*prev, last=True)
        emit_tail_b(prev[0], prev_cacc)

    nc.compile()
    return nc


def _make_in_maps(query, values, W1, b1, W2, b2, V):
    # bf16 data path for the big operands; fp32 for the bias/query path.
    valT = np.ascontiguousarray(
        values.transpose(0, 2, 1).astype(BF)
    )  # [B, H, S] bf16
    b12 = (b1 + b2).astype(np.float32)
    b12t = np.ascontiguousarray(b12.reshape(UC, P).T)
    vtp = np.zeros((P, UC, P), dtype=np.float32)
    vtp[:, :, 0] = V.reshape(UC, P).T
    vt = vtp.astype(BF)
    w1 = np.ascontiguousarray(W1.astype(BF))
    w2 = np.ascontiguousarray(W2.astype(BF))
    in_maps = []
    for c in range(N_CORES):
        sl = slice(c * BPC, (c + 1) * BPC)
        in_maps.append(
            {
                "qT": np.ascontiguousarray(query[sl].T).astype(BF),
                "valT": np.ascontiguousarray(valT[sl]),
                "w1": w1,
                "w2": w2,
                "b12t": b12t,
                "vt": vt,
            }
        )
    return in_maps


def _run(in_maps, trace=False, tmpdir=None):
    if trace:
        _install_ntff_shim()
    nc = _build_nc()
    res = run_bass_kernel_spmd(
        nc, in_maps, core_ids=list(range(N_CORES)), trace=trace, tmpdir=tmpdir
    )
    ctx = np.concatenate([res.results[c]["ctx"] for c in range(N_CORES)], axis=0)
    attn = np.concatenate([res.results[c]["attn"] for c in range(N_CORES)], axis=0)
    return (ctx, attn.reshape(B, S, 1)), res


def kernel(query, values, W1, b1, W2, b2, V, bv):
    # bv shifts every score of an example equally; softmax is shift-invariant,
    # so it affects neither output and is dropped.
    in_maps = _make_in_maps(
        np.asarray(query), np.asarray(values), np.asarray(W1), np.asarray(b1),
        np.asarray(W2), np.asarray(b2), np.asarray(V)
    )
    outs, _ = _run(in_maps, trace=False)
    return outs


# revision 21
# speedup vs baseline: 1.1341x; 1.0558x over previous
"""Bahdanau attention TRN2 kernel: 8-core data-parallel over batch.

Reference computation (B=64, S=2048, H=U=1024, fp32):
    q_proj = query @ W1 + b1                     # [B,U]
    v_proj = values @ W2 + b2                    # [B,S,U]
    score  = tanh(q_proj[:,None,:] + v_proj) @ V + bv   # [B,S,1]
    attn   = softmax(score, axis=1)              # [B,S,1]
    ctx    = sum(attn * values, axis=1)          # [B,H]
    returns (ctx, attn)

Per-core strategy (8 examples each, software-pipelined):
  - host pre-transposes values -> valT [B, H, S] (bf16) so the device streams
    it straight into the TensorEngine as the moving operand; contraction over
    H needs H on partitions for both operands.
  - v_projT tile [u_tile=128, s] = sum_k W2[k,u].T @ valT[k,s], bf16 matmuls
    (1 cyc/row, FWL weight loads), fp32 psum.
  - ScalarE evacuates psum with fused tanh + per-partition bias
    (q_projT + b1 + b2), bf16 out.
  - score = V.T @ tanh on PE -> psum row; ScalarE exp with accumulated row
    sum (no max subtraction needed: scores are O(1); softmax shift-invariance
    also lets us drop bv entirely).
  - attn = exp * (1/sum); context on DVE: ctx[h] = sum_s valT[h,s]*attn[s]
    with attn broadcast across partitions by gpsimd partition_broadcast.
  - example b's softmax/context tail is emitted inside example b+1's body so
    the TensorEngine never stalls on the DVE-side chain.
"""

import sys
import types
import functools
import contextlib

if "/opt/trn_rl_repo" not in sys.path:
    sys.path.insert(0, "/opt/trn_rl_repo")

import numpy as np
import ml_dtypes

import concourse.bass as bass
import concourse.tile as tile
from concourse import mybir, bacc
from concourse.bass_utils import run_bass_kernel_spmd
from concourse.masks import make_identity

N_CORES = 8
B, S, H, U = 64, 2048, 1024, 1024
BPC = B // N_CORES          # examples per core
P = 128                     # partitions
HC = H // P                 # h-chunks
UC = U // P                 # u-chunks
ST = 512                    # s-tile width (one fp32 psum bank)
NS = S // ST
F32 = mybir.dt.float32
BF16 = mybir.dt.bfloat16
AF = mybir.ActivationFunctionType
ALU = mybir.AluOpType
BF = ml_dtypes.bfloat16


def _install_ntff_shim():
    """antenv.axon_hooks is absent in this image; recreate it so
    run_bass_kernel_spmd(trace=True) can capture NTFF profiles."""
    try:
        from antenv import axon_hooks  # noqa: F401
        return
    except ImportError:
        pass
    import antenv
    mod = types.ModuleType("antenv.axon_hooks")
    mod._hook = None
    def set_axon_ntff_profile_hook(h):
        mod._hook = h
    def get_axon_ntff_profile_hook():
        return mod._hook
    mod.set_axon_ntff_profile_hook = set_axon_ntff_profile_hook
    mod.get_axon_ntff_profile_hook = get_axon_ntff_profile_hook
    sys.modules["antenv.axon_hooks"] = mod
    antenv.axon_hooks = mod
    try:
        from trn_agent_boot.trn_boot import _ntff_profile_via_ctypes
        set_axon_ntff_profile_hook(
            _ntff_profile_via_ctypes("/opt/axon/libaxon_pjrt.so")
        )
    except Exception:
        pass


@functools.lru_cache(maxsize=2)
def _build_nc():
    nc = bacc.Bacc("TRN2", target_bir_lowering=False, debug=False)

    qT = nc.declare_dram_parameter("qT", [H, BPC], BF16, isOutput=False)
    valT = nc.declare_dram_parameter("valT", [BPC, H, S], BF16, isOutput=False)
    w1 = nc.declare_dram_parameter("w1", [H, U], BF16, isOutput=False)
    w2 = nc.declare_dram_parameter("w2", [H, U], BF16, isOutput=False)
    b12t = nc.declare_dram_parameter("b12t", [P, UC], F32, isOutput=False)
    vt = nc.declare_dram_parameter("vt", [P, UC, P], BF16, isOutput=False)
    ctx_out = nc.declare_dram_parameter("ctx", [BPC, H], F32, isOutput=True)
    attn_out = nc.declare_dram_parameter("attn", [BPC, S], F32, isOutput=True)

    with tile.TileContext(nc) as tc, contextlib.ExitStack() as ex:
        consts = ex.enter_context(tc.tile_pool(name="consts", bufs=1))
        w2p = ex.enter_context(tc.tile_pool(name="w2p", bufs=1))
        valp = ex.enter_context(tc.tile_pool(name="valp", bufs=2 * HC))
        tanhp = ex.enter_context(tc.tile_pool(name="tanhp", bufs=8))
        junkp = ex.enter_context(tc.tile_pool(name="junkp", bufs=2))
        scoresp = ex.enter_context(tc.tile_pool(name="scoresp", bufs=2))
        bcp = ex.enter_context(tc.tile_pool(name="bcp", bufs=3))
        smallp = ex.enter_context(tc.tile_pool(name="smallp", bufs=2))
        psum_mm = ex.enter_context(tc.tile_pool(name="psum_mm", bufs=5, space="PSUM"))
        psum_sc = ex.enter_context(tc.tile_pool(name="psum_sc", bufs=3, space="PSUM"))

        identb = consts.tile([P, P], BF16)
        make_identity(nc, identb)
        b12sb = consts.tile([P, UC], F32)
        nc.sync.dma_start(out=b12sb, in_=b12t.ap())
        vtsb = consts.tile([P, UC, P], BF16)
        nc.sync.dma_start(out=vtsb, in_=vt.ap())
        qsb = consts.tile([P, HC, BPC], BF16)
        nc.sync.dma_start(out=qsb, in_=qT.ap().rearrange("(hc p) b -> p hc b", p=P))

        # q_projT[u, b] (+ b1 + b2), laid out [P, UC, BPC]; fp32, startup-only.
        qproj = consts.tile([P, UC, BPC], F32)
        with tc.tile_pool(name="w1p", bufs=1) as w1p:
            w1sb = w1p.tile([P, HC, U], BF16)
            nc.sync.dma_start(
                out=w1sb, in_=w1.ap().rearrange("(hc p) u -> p hc u", p=P)
            )
            for m in range(UC):
                pq = psum_mm.tile([P, BPC], F32, tag="mm")
                for k in range(HC):
                    nc.tensor.matmul(
                        pq,
                        lhsT=w1sb[:, k, m * P : (m + 1) * P],
                        rhs=qsb[:, k, :],
                        start=(k == 0),
                        stop=(k == HC - 1),
                    )
                nc.vector.tensor_scalar_add(qproj[:, m, :], pq, b12sb[:, m : m + 1])

        w2sb = w2p.tile([P, HC, U], BF16)
        nc.sync.dma_start(out=w2sb, in_=w2.ap().rearrange("(hc p) u -> p hc u", p=P))

        def emit_loads(b):
            vts = []
            for k in range(HC):
                t = valp.tile([P, S], BF16, tag="valT")
                nc.sync.dma_start(out=t, in_=valT.ap()[b, k * P : (k + 1) * P, :])
                vts.append(t)
            return vts

        def emit_exp(ps, si, sc, lsum):
            nc.scalar.activation(
                out=sc[:, si * ST : (si + 1) * ST], in_=ps[0:1, :], func=AF.Exp,
                accum_out=lsum[:, si : si + 1],
            )

        def emit_compute_si(b, si, vts, sc, lsum):
            s0 = si * ST
            tts = []
            for m in range(UC):
                pv = psum_mm.tile([P, ST], F32, tag="mm")
                for k in range(HC):
                    nc.tensor.matmul(
                        pv,
                        lhsT=w2sb[:, k, m * P : (m + 1) * P],
                        rhs=vts[k][:, s0 : s0 + ST],
                        start=(k == 0),
                        stop=(k == HC - 1),
                    )
                th = tanhp.tile([P, ST], BF16, tag="tanh")
                nc.scalar.activation(
                    out=th, in_=pv, func=AF.Tanh,
                    bias=qproj[:, m, b : b + 1], scale=1.0,
                )
                tts.append(th)
            ps = psum_sc.tile([P, ST], F32, tag="sc")
            for m in range(UC):
                nc.tensor.matmul(
                    ps, lhsT=vtsb[:, m, :], rhs=tts[m],
                    start=(m == 0), stop=(m == UC - 1),
                )
            return ps

        def emit_online_ctx(vts, sc, si, cacc3):
            """per-s-tile context accumulation with UNNORMALIZED exp scores
            (softmax scale applied once at example end) — no PE involvement."""
            bc = bcp.tile([P, ST], BF16, tag="bc")
            nc.gpsimd.partition_broadcast(bc, sc[:, si * ST : (si + 1) * ST])
            for k in range(HC):
                junk = junkp.tile([P, ST], BF16, tag="junk")
                nc.vector.tensor_mul(junk, vts[k][:, si * ST : (si + 1) * ST], bc)
                nc.vector.reduce_sum(
                    out=cacc3[:, k, si : si + 1], in_=junk,
                    axis=mybir.AxisListType.X,
                )

        def emit_example_end(b, sc, lsum, cacc3):
            """softmax normalization of attn + context scale (no PE)."""
            ltot = smallp.tile([1, 1], F32, tag="ltot")
            nc.vector.reduce_sum(out=ltot, in_=lsum, axis=mybir.AxisListType.X)
            rinv = smallp.tile([1, 1], F32, tag="rinv")
            nc.vector.reciprocal(rinv, ltot)
            nc.vector.tensor_scalar_mul(sc, sc, rinv)
            nc.gpsimd.dma_start(out=attn_out.ap()[b : b + 1, :], in_=sc)
            rinv_bc = smallp.tile([P, 1], F32, tag="rinvbc")
            nc.gpsimd.partition_broadcast(rinv_bc, rinv)
            cacc = smallp.tile([P, HC], F32, tag="cacc")
            nc.vector.reduce_sum(
                out=cacc.rearrange("p (hc one) -> p hc one", one=1),
                in_=cacc3, axis=mybir.AxisListType.X,
            )
            nc.vector.tensor_scalar_mul(cacc, cacc, rinv_bc)
            return cacc

        def emit_tail_b(b, cacc):
            """ctx transpose + writeback (one tiny bf16 PE op, placed late)."""
            caccb = smallp.tile([P, HC], BF16, tag="caccb")
            nc.vector.tensor_copy(caccb, cacc)
            pt = psum_sc.tile([HC, P], BF16, tag="sc")
            nc.tensor.transpose(pt, caccb, identb)
            ctxT = smallp.tile([HC, P], BF16, tag="ctxT")
            nc.vector.tensor_copy(ctxT, pt)
            nc.gpsimd.dma_start(
                out=ctx_out.ap()[b : b + 1, :].rearrange(
                    "one (hc p) -> hc (one p)", p=P
                ),
                in_=ctxT,
            )

        # software pipeline: exp + online context for s-tile si are emitted
        # one s-tile later; example-end softmax/scale lands in the next
        # example's body so PE never waits on the DVE chain.
        pend = None          # (vts, sc, si, lsum, ps, cacc3)
        pend_end = None      # (b, sc, lsum, cacc3)
        pend_cacc = None     # (b, cacc)
        for b in range(BPC):
            vts = emit_loads(b)
            sc = scoresp.tile([1, S], BF16, tag="scores")
            lsum = smallp.tile([1, NS], F32, tag="lsum")
            cacc3 = smallp.tile([P, HC, NS], F32, tag="cacc3")
            for si in range(NS):
                ps = emit_compute_si(b, si, vts, sc, lsum)
                if pend is not None:
                    pvts, psc, psi, plsum, pps, pcacc3 = pend
                    emit_exp(pps, psi, psc, plsum)
                    emit_online_ctx(pvts, psc, psi, pcacc3)
                    if psi == NS - 1:
                        pend_end = (pend_b, psc, plsum, pcacc3)
                if pend_end is not None and si == 1:
                    pend_cacc = (pend_end[0], emit_example_end(*pend_end))
                    pend_end = None
                pend = (vts, sc, si, lsum, ps, cacc3)
                pend_b = b
            if pend_cacc is not None:
                emit_tail_b(*pend_cacc)
                pend_cacc = None
        # flush: last example's final s-tile + end
        pvts, psc, psi, plsum, pps, pcacc3 = pend
        emit_exp(pps, psi, psc, plsum)
        emit_online_ctx(pvts, psc, psi, pcacc3)
        cacc = emit_example_end(BPC - 1, psc, plsum, pcacc3)
        emit_tail_b(BPC - 1, cacc)

    nc.compile()
    return nc


def _make_in_maps(query, values, W1, b1, W2, b2, V):
    # bf16 data path for the big operands; fp32 for the bias/query path.
    valT = np.ascontiguousarray(
        values.transpose(0, 2, 1).astype(BF)
    )  # [B, H, S] bf16
    b12 = (b1 + b2).astype(np.float32)
    b12t = np.ascontiguousarray(b12.reshape(UC, P).T)
    vtp = np.zeros((P, UC, P), dtype=np.float32)
    vtp[:, :, 0] = V.reshape(UC, P).T
    vt = vtp.astype(BF)
    w1 = np.ascontiguousarray(W1.astype(BF))
    w2 = np.ascontiguousarray(W2.astype(BF))
    in_maps = []
    for c in range(N_CORES):
        sl = slice(c * BPC, (c + 1) * BPC)
        in_maps.append(
            {
                "qT": np.ascontiguousarray(query[sl].T).astype(BF),
                "valT": np.ascontiguousarray(valT[sl]),
                "w1": w1,
                "w2": w2,
                "b12t": b12t,
                "vt": vt,
            }
        )
    return in_maps


def _run(in_maps, trace=False, tmpdir=None):
    if trace:
        _install_ntff_shim()
    nc = _build_nc()
    res = run_bass_kernel_spmd(
        nc, in_maps, core_ids=list(range(N_CORES)), trace=trace, tmpdir=tmpdir
    )
    ctx = np.concatenate([res.results[c]["ctx"] for c in range(N_CORES)], axis=0)
    attn = np.concatenate([res.results[c]["attn"] for c in range(N_CORES)], axis=0)
    return (ctx, attn.reshape(B, S, 1)), res


def kernel(query, values, W1, b1, W2, b2, V, bv):
    # bv shifts every score of an example equally; softmax is shift-invariant,
    # so it affects neither output and is dropped.
    in_maps = _make_in_maps(
        np.asarray(query), np.asarray(values), np.asarray(W1), np.asarray(b1),
        np.asarray(W2), np.asarray(b2), np.asarray(V)
    )
    outs, _ = _run(in_maps, trace=False)
    return outs
